# revision 1
# baseline (speedup 1.0000x reference)
"""Masked attention-aggregator kernel for Trainium2 (8 NeuronCores, SPMD).

Reference computation (B=16, N=2048, D=128, DQ=64), all fp32:
    q = x @ Wq.T + bq                      [B, N, DQ]
    k = x @ Wk.T + bk                      [B, N, DQ]
    s = (k @ q.T) / sqrt(DQ)               [B, N, N]   (s[b,n,m] = k[n].q[m])
    w = softmax(s + (mask[m]>0 ? 0 : -1e9), axis=m)
    out = w @ x                            [B, N, D]

Strategy: data-parallel over batch (2 batches per core).  Per batch, a
flash-style streaming attention that never materializes [N, N] anywhere:

  * The masked axis (m) indexes the *queries* side.  Masked m-columns get
    softmax weight exactly 0, so on the host we COMPACT the m axis: gather
    the unmasked rows of x per batch, pad to a multiple of 128.  Padded
    rows are killed by a -30000 additive penalty (exp -> 0), so they
    contribute to neither numerator nor denominator.  The penalty rides a
    65th contraction row of the scores matmul: queriesT gets the penalty
    row, keysT gets an all-ones row.
  * Scores are computed transposed, ST[m, n] = q_s[m].k[n], with m on PSUM
    partitions, so E^T = exp(ST) lands in SBUF already in the right layout
    to be the stationary operand source for both the numerator
    (x_c^T @ E^T -> out^T[d, n]) and the denominator
    (ones^T @ E^T -> den replicated over all 128 partitions, so the final
    divide is a plain elementwise op, no partition broadcast needed).
  * The 1/sqrt(DQ) scale is folded into Wq on the host (exact: 0.125 is a
    power of two).
  * All big matmuls run as float32r (full PE rate at free-dim >= 256).
  * The output is produced transposed ([D, N] per batch, contiguous
    per-partition rows -> one fast DMA) and un-transposed on the host.
"""

import math
import os

import numpy as np

B, N, D, DQ = 16, 2048, 128, 64
NCORES = 8
BPC = B // NCORES  # batches per core

_cache = {}


def _build_program(mcp: int, reps: int = 1, NG: int = 1024, psum_bufs: int = 2,
                   pool_split: bool = False, et_bufs: int = 3,
                   store_sync: bool = False):
    """Build the per-core Bass program for a compacted/padded m-size of mcp."""
    import concourse.bass as bass
    import concourse.tile as tile
    from concourse import bacc, mybir

    f32 = mybir.dt.float32
    f32r = mybir.dt.float32r
    mc = mcp // 128  # number of m chunks
    ngroups = N // NG

    nc = bacc.Bacc("TRN2", target_bir_lowering=False, debug=False, num_devices=1)

    xt = nc.dram_tensor("xt", [BPC, D, N], f32r, kind="ExternalInput").ap()
    xtc = nc.dram_tensor("xtc", [BPC, D, mcp], f32r, kind="ExternalInput").ap()
    xcb = nc.dram_tensor("xcb", [BPC, 128, mc * D], f32r, kind="ExternalInput").ap()
    penrow = nc.dram_tensor("penrow", [BPC, 1, mcp], f32r, kind="ExternalInput").ap()
    onerow = nc.dram_tensor("onerow", [1, N], f32r, kind="ExternalInput").ap()
    wqt = nc.dram_tensor("wqt", [D, DQ], f32r, kind="ExternalInput").ap()
    wkt = nc.dram_tensor("wkt", [D, DQ], f32r, kind="ExternalInput").ap()
    bqs = nc.dram_tensor("bqs", [DQ, 1], f32, kind="ExternalInput").ap()
    bks = nc.dram_tensor("bks", [DQ, 1], f32, kind="ExternalInput").ap()
    onesd = nc.dram_tensor("onesd", [128, 128], f32r, kind="ExternalInput").ap()
    out = nc.dram_tensor("out", [BPC, D, N], f32, kind="ExternalOutput").ap()

    with tile.TileContext(nc) as tc:
        with (
            tc.tile_pool(name="singles", bufs=1) as singles,
            tc.tile_pool(name="xtp", bufs=2) as xtp,
            tc.tile_pool(name="xtcp", bufs=2) as xtcp,
            tc.tile_pool(name="xcp", bufs=2) as xcp,
            tc.tile_pool(name="qtp", bufs=2) as qtp,
            tc.tile_pool(name="ktp", bufs=2) as ktp,
            tc.tile_pool(name="etp", bufs=et_bufs) as etp,
            tc.tile_pool(name="nrmp", bufs=2) as nrmp,
            tc.tile_pool(name="eap", bufs=2) as eap,
            tc.tile_pool(name="st", bufs=psum_bufs, space="PSUM") as stp,
            tc.tile_pool(name="oa", bufs=psum_bufs, space="PSUM") as oap,
        ):
            wq_sb = singles.tile([D, DQ], f32r)
            nc.sync.dma_start(wq_sb[:], wqt[:])
            wk_sb = singles.tile([D, DQ], f32r)
            nc.sync.dma_start(wk_sb[:], wkt[:])
            bq_sb = singles.tile([DQ, 1], f32)
            nc.sync.dma_start(bq_sb[:], bqs[:])
            bk_sb = singles.tile([DQ, 1], f32)
            nc.sync.dma_start(bk_sb[:], bks[:])
            ones = singles.tile([128, 128], f32r)
            nc.sync.dma_start(ones[:], onesd[:])

            def body():
              for b in range(BPC):
                # ---- loads ----
                xt_t = xtp.tile([D, N], f32r, tag="xt")
                nc.sync.dma_start(xt_t[:], xt[b])
                xtc_t = xtcp.tile([D, mcp], f32r, tag="xtc")
                nc.sync.dma_start(xtc_t[:], xtc[b])
                xcb_t = xcp.tile([128, mc * D], f32r, tag="xc")
                nc.sync.dma_start(xcb_t[:], xcb[b])

                # ---- projections (into [dq+1, m] / [dq+1, n] layout) ----
                def project(dst, w_sb, src, src_w, bias_sb):
                    for j0 in range(0, src_w, NG):
                        span = min(NG, src_w - j0)
                        pp = stp.tile([128, NG], f32, tag="st")
                        for j in range(0, span, 512):
                            jw = min(512, span - j)
                            nc.tensor.matmul(pp[0:DQ, j:j + jw], w_sb[:],
                                             src[:, j0 + j:j0 + j + jw],
                                             start=True, stop=True)
                        nc.vector.tensor_scalar_add(dst[0:DQ, j0:j0 + span],
                                                    pp[0:DQ, 0:span], bias_sb[:])

                qt_t = qtp.tile([DQ + 1, mcp], f32r, tag="qt")
                nc.gpsimd.dma_start(qt_t[DQ:DQ + 1, :], penrow[b])
                project(qt_t, wq_sb, xtc_t, mcp, bq_sb)
                kt_t = ktp.tile([DQ + 1, N], f32r, tag="kt")
                nc.gpsimd.dma_start(kt_t[DQ:DQ + 1, :], onerow[:])
                project(kt_t, wk_sb, xt_t, N, bk_sb)

                # ---- attention over n-groups ----
                for g in range(ngroups):
                    oa = oap.tile([128, NG], f32, tag="oa")
                    eacc = eap.tile([128, NG], f32r, tag="eacc")
                    for m in range(mc):
                        st = stp.tile([128, NG], f32, tag="st")
                        for h in range(NG // 512):
                            nc.tensor.matmul(
                                st[:, h * 512:(h + 1) * 512],
                                qt_t[:, m * 128:(m + 1) * 128],
                                kt_t[:, g * NG + h * 512: g * NG + (h + 1) * 512],
                                start=True, stop=True)
                        et = etp.tile([128, NG], f32r, tag="et")
                        nc.scalar.activation(et[:], st[:],
                                             mybir.ActivationFunctionType.Exp)
                        first, last = (m == 0), (m == mc - 1)
                        for h in range(NG // 512):
                            hs = slice(h * 512, (h + 1) * 512)
                            nc.tensor.matmul(oa[:, hs],
                                             xcb_t[:, m * D:(m + 1) * D],
                                             et[:, hs], start=first, stop=last)
                        half = NG // 2
                        if first:
                            if pool_split:
                                nc.vector.tensor_copy(eacc[:, :half], et[:, :half])
                                nc.gpsimd.tensor_copy(eacc[:, half:], et[:, half:])
                            else:
                                nc.vector.tensor_copy(eacc[:], et[:])
                        elif pool_split:
                            nc.vector.tensor_add(eacc[:, :half], eacc[:, :half],
                                                 et[:, :half])
                            nc.gpsimd.tensor_add(eacc[:, half:], eacc[:, half:],
                                                 et[:, half:])
                        else:
                            nc.vector.tensor_add(eacc[:], eacc[:], et[:])
                    # den replicated over partitions via ones.T @ eacc
                    dn = stp.tile([128, NG], f32, tag="st")
                    for h in range(NG // 512):
                        hs = slice(h * 512, (h + 1) * 512)
                        nc.tensor.matmul(dn[:, hs], ones[:], eacc[:, hs],
                                         start=True, stop=True)
                    rden = nrmp.tile([128, NG], f32, tag="rden")
                    nc.vector.reciprocal(rden[:], dn[:])
                    nrm = nrmp.tile([128, NG], f32, tag="nrm")
                    nc.vector.tensor_mul(nrm[:], oa[:], rden[:])
                    if store_sync:
                        nc.sync.dma_start(out[b][:, g * NG:(g + 1) * NG], nrm[:])
                    else:
                        nc.gpsimd.dma_start(out[b][:, g * NG:(g + 1) * NG], nrm[:])

            if reps > 1:
                with tc.For_i(0, reps, 1):
                    body()
            else:
                body()

    nc.compile()
    return nc


def _prep(x, mask, Wq, bq, Wk, bk):
    """Host-side prep: compaction, transposes, sharding.  Returns (in_maps, mcp)."""
    x = np.ascontiguousarray(np.asarray(x, dtype=np.float32))
    mask = np.asarray(mask)
    Wq = np.asarray(Wq, dtype=np.float32)
    bq = np.asarray(bq, dtype=np.float32)
    Wk = np.asarray(Wk, dtype=np.float32)
    bk = np.asarray(bk, dtype=np.float32)

    scale = np.float32(1.0 / math.sqrt(DQ))

    # host-side compaction of the masked (aggregated) axis
    keep = [np.nonzero(mask[b] > 0)[0] for b in range(B)]
    counts = [len(k) for k in keep]
    mcap = max(max(counts), 1)
    mcp = ((mcap + 127) // 128) * 128
    mc = mcp // 128

    xc = np.zeros((B, mcp, D), dtype=np.float32)
    pen = np.full((B, 1, mcp), -30000.0, dtype=np.float32)
    for b in range(B):
        cnt = counts[b]
        if cnt:
            xc[b, :cnt] = x[b, keep[b]]
            pen[b, 0, :cnt] = 0.0

    xt = np.ascontiguousarray(x.transpose(0, 2, 1))          # [B, D, N]
    xtc = np.ascontiguousarray(xc.transpose(0, 2, 1))        # [B, D, mcp]
    # chunk-major xc: xcb[b, p, m*D + d] = xc[b, m*128 + p, d]
    xcb = np.ascontiguousarray(
        xc.reshape(B, mc, 128, D).transpose(0, 2, 1, 3).reshape(B, 128, mc * D))
    wqt = np.ascontiguousarray((Wq * scale).T)               # [D, DQ]
    wkt = np.ascontiguousarray(Wk.T)                         # [D, DQ]
    bqs = np.ascontiguousarray((bq * scale).reshape(DQ, 1))
    bks = np.ascontiguousarray(bk.reshape(DQ, 1))
    ones_mat = np.ones((128, 128), dtype=np.float32)
    one_row = np.ones((1, N), dtype=np.float32)

    in_maps = []
    for c in range(NCORES):
        s = slice(c * BPC, (c + 1) * BPC)
        in_maps.append({
            "xt": xt[s], "xtc": xtc[s], "xcb": xcb[s], "penrow": pen[s],
            "wqt": wqt, "wkt": wkt, "bqs": bqs, "bks": bks,
            "onesd": ones_mat, "onerow": one_row,
        })
    return in_maps, mcp


def kernel(x, mask, Wq, bq, Wk, bk):
    from concourse import bass_utils

    in_maps, mcp = _prep(x, mask, Wq, bq, Wk, bk)

    if mcp not in _cache:
        _cache[mcp] = _build_program(mcp)
    nc = _cache[mcp]

    res = bass_utils.run_bass_kernel_spmd(
        nc, in_maps, core_ids=list(range(NCORES)),
        trace=bool(os.environ.get("BASS_TRACE")),
    )
    kernel._last_results = res

    out_t = np.concatenate([res.results[c]["out"] for c in range(NCORES)], axis=0)
    return np.ascontiguousarray(out_t.transpose(0, 2, 1)).astype(np.float32)

